# revision 1
# baseline (speedup 1.0000x reference)
"""MoE MLP (LayerNorm -> top-2 gate -> 4-expert 2-layer GELU MLP) on 8 NeuronCores.

Strategy:
  Phase 1 (token-parallel, 8 x 576 tokens): fp32 LayerNorm, PE transpose to
    get xn^T, fp32 gate matmul, softmax + top-2 + renormalized combine
    weights, and per-expert assignment counts / prob sums (for the aux loss).
  Host: all-to-all token dispatch - gather each expert's tokens (columns of
    xn^T), cast to bf16, split each expert across 2 cores.
  Phase 2 (expert-parallel, 4 experts x 2 token halves, capacity 1280/core):
    weights-stationary bf16 matmuls with fp32 accumulate;
    h = GELU(W1.T x + b1) fused on the scalar engine; o = W2.T h + b2,
    scaled by the combine weight on-chip.
  Host: scatter-add the (<=2) expert contributions per token; aux loss from
    the phase-1 stats.
"""
import numpy as np
import ml_dtypes

import concourse.bass as bass  # noqa: F401  (bass must import before tile)
import concourse.mybir as mybir
import concourse.tile as tile
from concourse import bacc
from concourse.masks import make_identity
from concourse.bass_utils import run_bass_kernel_spmd

F32 = mybir.dt.float32
BF16 = mybir.dt.bfloat16
AF = mybir.ActivationFunctionType
ALU = mybir.AluOpType
AX = mybir.AxisListType
BF16_NP = ml_dtypes.bfloat16

B, N_TOK, D, H, E, K = 8, 576, 1024, 2048, 4, 2
T = B * N_TOK          # 4608 tokens
T_LOC = N_TOK          # tokens per core in phase 1
P = 128
D_T = D // P           # 8
H_T = H // P           # 16
N_TT = (T_LOC + P - 1) // P   # 5 t-tiles (4 full + 1 of 64)
LN_EPS = 1e-6
CAP = 1280             # phase-2 token capacity per core (max expert load 2357 -> 1179/core)
TBS = [(0, 512), (512, 512), (1024, 256)]
N_CORES = 8

_CACHE = {}


def _build_phase1(reps=1):
    nc = bacc.Bacc("TRN2", target_bir_lowering=False, debug=False,
                   num_devices=N_CORES)
    x = nc.dram_tensor("x", [T_LOC, D], F32, kind="ExternalInput")
    wg = nc.dram_tensor("wg", [D, E], F32, kind="ExternalInput")
    bg = nc.dram_tensor("bg", [1, E], F32, kind="ExternalInput")
    gamma = nc.dram_tensor("gamma", [1, D], F32, kind="ExternalInput")
    beta = nc.dram_tensor("beta", [1, D], F32, kind="ExternalInput")
    xnt = nc.dram_tensor("xnt", [D, T_LOC], F32, kind="ExternalOutput")
    combine = nc.dram_tensor("combine", [T_LOC, E], F32, kind="ExternalOutput")
    stats = nc.dram_tensor("stats", [2, E], F32, kind="ExternalOutput")

    with tile.TileContext(nc) as tc:
        with (
            tc.tile_pool(name="const", bufs=1) as cpool,
            tc.tile_pool(name="xin", bufs=3) as xpool,
            tc.tile_pool(name="ln", bufs=3) as lnpool,
            tc.tile_pool(name="small", bufs=4) as spool,
            tc.tile_pool(name="xnt", bufs=1) as xntpool,
            tc.tile_pool(name="ps", bufs=2, space="PSUM") as pspool,
            tc.tile_pool(name="psg", bufs=2, space="PSUM") as psgpool,
        ):
            ident = cpool.tile([P, P], F32)
            make_identity(nc, ident[:])
            ones = cpool.tile([P, 1], F32)
            nc.vector.memset(ones[:], 1.0)
            epst = cpool.tile([P, 1], F32)
            nc.vector.memset(epst[:], LN_EPS)
            wgsb = cpool.tile([P, D_T, E], F32)
            nc.sync.dma_start(wgsb[:], wg.rearrange("(dt p) e -> p dt e", p=P))
            g_row = cpool.tile([1, D], F32)
            nc.sync.dma_start(g_row[:], gamma[:])
            b_row = cpool.tile([1, D], F32)
            nc.sync.dma_start(b_row[:], beta[:])
            bg_row = cpool.tile([1, E], F32)
            nc.sync.dma_start(bg_row[:], bg[:])
            g_bc = cpool.tile([P, D], F32)
            nc.gpsimd.partition_broadcast(g_bc[:], g_row[:])
            b_bc = cpool.tile([P, D], F32)
            nc.gpsimd.partition_broadcast(b_bc[:], b_row[:])
            bg_bc = cpool.tile([P, E], F32)
            nc.gpsimd.partition_broadcast(bg_bc[:], bg_row[:])

            def body(_iv=None):
                xnt_sb = xntpool.tile([P, D_T, T_LOC], F32, tag="xnt_sb")
                cnt_acc = spool.tile([1, E], F32, tag="cnt_acc")
                nc.vector.memset(cnt_acc[:], 0.0)
                prb_acc = spool.tile([1, E], F32, tag="prb_acc")
                nc.vector.memset(prb_acc[:], 0.0)

                for it in range(N_TT):
                    t0 = it * P
                    tp = min(P, T_LOC - t0)
                    xt = xpool.tile([P, D], F32, tag="xt")
                    nc.sync.dma_start(xt[:tp], x[t0:t0 + tp, :])
                    xv = xt[:tp]

                    # LayerNorm (two-pass mean/var, fp32)
                    s1 = spool.tile([P, 1], F32, tag="s1")
                    nc.vector.reduce_sum(s1[:tp], xv, axis=AX.X)
                    nmu = spool.tile([P, 1], F32, tag="nmu")
                    nc.vector.tensor_scalar_mul(nmu[:tp], s1[:tp], -1.0 / D)
                    xc = lnpool.tile([P, D], F32, tag="xc")
                    nc.scalar.activation(xc[:tp], xv, AF.Identity,
                                         bias=nmu[:tp], scale=1.0)
                    sq = lnpool.tile([P, D], F32, tag="sq")
                    nc.vector.tensor_mul(sq[:tp], xc[:tp], xc[:tp])
                    s2 = spool.tile([P, 1], F32, tag="s2")
                    nc.vector.reduce_sum(s2[:tp], sq[:tp], axis=AX.X)
                    var = spool.tile([P, 1], F32, tag="var")
                    nc.vector.tensor_scalar_mul(var[:tp], s2[:tp], 1.0 / D)
                    std = spool.tile([P, 1], F32, tag="std")
                    nc.scalar.activation(std[:tp], var[:tp], AF.Sqrt,
                                         bias=epst[:tp], scale=1.0)
                    rstd = spool.tile([P, 1], F32, tag="rstd")
                    nc.vector.reciprocal(rstd[:tp], std[:tp])
                    xn = lnpool.tile([P, D], F32, tag="xn")
                    nc.vector.tensor_scalar_mul(xn[:tp], xc[:tp], rstd[:tp])
                    nc.vector.tensor_mul(xn[:tp], xn[:tp], g_bc[:tp])
                    nc.vector.tensor_add(xn[:tp], xn[:tp], b_bc[:tp])

                    # transpose to xn^T
                    for dt in range(D_T):
                        pst = pspool.tile([P, P], F32, tag="pst")
                        nc.tensor.transpose(pst[:, :tp],
                                            xn[:tp, dt * P:(dt + 1) * P],
                                            ident[:tp, :tp])
                        nc.vector.tensor_copy(xnt_sb[:, dt, t0:t0 + tp],
                                              pst[:, :tp])

                    # gate logits (fp32 matmul)
                    psg = psgpool.tile([P, E], F32, tag="psg")
                    for dt in range(D_T):
                        nc.tensor.matmul(psg[:tp], xnt_sb[:, dt, t0:t0 + tp],
                                         wgsb[:, dt], start=(dt == 0),
                                         stop=(dt == D_T - 1))
                    logit = spool.tile([P, E], F32, tag="logit")
                    nc.vector.tensor_add(logit[:tp], psg[:tp], bg_bc[:tp])

                    # softmax over experts (free dim)
                    mx = spool.tile([P, 1], F32, tag="mx")
                    nc.vector.reduce_max(mx[:tp], logit[:tp], axis=AX.X)
                    nmx = spool.tile([P, 1], F32, tag="nmx")
                    nc.vector.tensor_scalar_mul(nmx[:tp], mx[:tp], -1.0)
                    el = spool.tile([P, E], F32, tag="el")
                    nc.scalar.activation(el[:tp], logit[:tp], AF.Exp,
                                         bias=nmx[:tp], scale=1.0)
                    ssum = spool.tile([P, 1], F32, tag="ssum")
                    nc.vector.reduce_sum(ssum[:tp], el[:tp], axis=AX.X)
                    rsum = spool.tile([P, 1], F32, tag="rsum")
                    nc.vector.reciprocal(rsum[:tp], ssum[:tp])
                    prob = spool.tile([P, E], F32, tag="prob")
                    nc.vector.tensor_scalar_mul(prob[:tp], el[:tp], rsum[:tp])

                    # top-2 selection + renormalized combine weights
                    m1 = spool.tile([P, 1], F32, tag="m1")
                    nc.vector.reduce_max(m1[:tp], prob[:tp], axis=AX.X)
                    eq = spool.tile([P, E], F32, tag="eq")
                    nc.vector.tensor_scalar(eq[:tp], prob[:tp], m1[:tp], -2.0,
                                            ALU.is_equal, ALU.mult)
                    pm = spool.tile([P, E], F32, tag="pm")
                    nc.vector.tensor_add(pm[:tp], prob[:tp], eq[:tp])
                    m2 = spool.tile([P, 1], F32, tag="m2")
                    nc.vector.reduce_max(m2[:tp], pm[:tp], axis=AX.X)
                    mask = spool.tile([P, E], F32, tag="mask")
                    nc.vector.tensor_scalar(mask[:tp], prob[:tp], m2[:tp], None,
                                            ALU.is_ge)
                    den = spool.tile([P, 1], F32, tag="den")
                    nc.vector.tensor_add(den[:tp], m1[:tp], m2[:tp])
                    rden = spool.tile([P, 1], F32, tag="rden")
                    nc.vector.reciprocal(rden[:tp], den[:tp])
                    cmb = spool.tile([P, E], F32, tag="cmb")
                    nc.vector.tensor_mul(cmb[:tp], prob[:tp], mask[:tp])
                    nc.vector.tensor_scalar_mul(cmb[:tp], cmb[:tp], rden[:tp])
                    nc.sync.dma_start(combine[t0:t0 + tp, :], cmb[:tp])

                    # aux-loss stats: column sums via ones-matmul
                    psc = psgpool.tile([1, E], F32, tag="psc")
                    nc.tensor.matmul(psc[:], ones[:tp], mask[:tp],
                                     start=True, stop=True)
                    nc.vector.tensor_add(cnt_acc[:], cnt_acc[:], psc[:])
                    psp = psgpool.tile([1, E], F32, tag="psp")
                    nc.tensor.matmul(psp[:], ones[:tp], prob[:tp],
                                     start=True, stop=True)
                    nc.vector.tensor_add(prb_acc[:], prb_acc[:], psp[:])

                for dt in range(D_T):
                    nc.sync.dma_start(xnt[dt * P:(dt + 1) * P, :], xnt_sb[:, dt])
                nc.sync.dma_start(stats[0:1, :], cnt_acc[:])
                nc.sync.dma_start(stats[1:2, :], prb_acc[:])

            if reps == 1:
                body()
            else:
                with tc.For_i(0, reps, 1) as _i:
                    body(_i)
    nc.compile()
    return nc


def _build_phase2(reps=1):
    nc = bacc.Bacc("TRN2", target_bir_lowering=False, debug=False,
                   num_devices=N_CORES)
    xt = nc.dram_tensor("xt", [D, CAP], BF16, kind="ExternalInput")
    w1 = nc.dram_tensor("w1", [D, H], BF16, kind="ExternalInput")
    b1 = nc.dram_tensor("b1", [H], F32, kind="ExternalInput")
    w2 = nc.dram_tensor("w2", [H, H], BF16, kind="ExternalInput")
    b2 = nc.dram_tensor("b2", [H], F32, kind="ExternalInput")
    sc = nc.dram_tensor("sc", [1, CAP], F32, kind="ExternalInput")
    ot = nc.dram_tensor("ot", [H, CAP], F32, kind="ExternalOutput")

    with tile.TileContext(nc) as tc:
        with (
            tc.tile_pool(name="const", bufs=1) as cpool,
            tc.tile_pool(name="w", bufs=1) as wpool,
            tc.tile_pool(name="h", bufs=2) as hpool,
            tc.tile_pool(name="tmp", bufs=3) as tpool,
            tc.tile_pool(name="ps", bufs=6, space="PSUM") as pspool,
        ):
            b1sb = cpool.tile([P, H_T], F32)
            nc.sync.dma_start(b1sb[:], b1.rearrange("(ht p) -> p ht", p=P))
            b2sb = cpool.tile([P, H_T], F32)
            nc.sync.dma_start(b2sb[:], b2.rearrange("(ht p) -> p ht", p=P))
            sc_row = cpool.tile([1, CAP], F32)
            nc.sync.dma_start(sc_row[:], sc[:])
            scb = cpool.tile([P, CAP], F32)
            nc.gpsimd.partition_broadcast(scb[:], sc_row[:])

            def body(_iv=None):
                xtsb = wpool.tile([P, D_T, CAP], BF16, tag="xtsb")
                nc.sync.dma_start(xtsb[:], xt.rearrange("(dt p) t -> p dt t", p=P))
                w1sb = wpool.tile([P, D_T, H], BF16, tag="w1sb")
                nc.sync.dma_start(w1sb[:], w1.rearrange("(dt p) h -> p dt h", p=P))
                w2sb = wpool.tile([P, H_T, H], BF16, tag="w2sb")
                nc.sync.dma_start(w2sb[:], w2.rearrange("(ht p) o -> p ht o", p=P))

                for (t0, tn) in TBS:
                    hsb = hpool.tile([P, H_T, 512], BF16, tag="hsb")
                    for ho in range(H_T):
                        ps = pspool.tile([P, 512], F32, tag="ps")
                        for dt in range(D_T):
                            nc.tensor.matmul(ps[:, :tn],
                                             w1sb[:, dt, ho * P:(ho + 1) * P],
                                             xtsb[:, dt, t0:t0 + tn],
                                             start=(dt == 0),
                                             stop=(dt == D_T - 1))
                        nc.scalar.activation(hsb[:, ho, :tn], ps[:, :tn],
                                             AF.Gelu, bias=b1sb[:, ho:ho + 1],
                                             scale=1.0)
                    for oo in range(H_T):
                        ps2 = pspool.tile([P, 512], F32, tag="ps")
                        for ht in range(H_T):
                            nc.tensor.matmul(ps2[:, :tn],
                                             w2sb[:, ht, oo * P:(oo + 1) * P],
                                             hsb[:, ht, :tn],
                                             start=(ht == 0),
                                             stop=(ht == H_T - 1))
                        tmp = tpool.tile([P, 512], F32, tag="tmp")
                        nc.scalar.activation(tmp[:, :tn], ps2[:, :tn],
                                             AF.Identity,
                                             bias=b2sb[:, oo:oo + 1], scale=1.0)
                        nc.vector.tensor_mul(tmp[:, :tn], tmp[:, :tn],
                                             scb[:, t0:t0 + tn])
                        nc.sync.dma_start(ot[oo * P:(oo + 1) * P, t0:t0 + tn],
                                          tmp[:, :tn])

            if reps == 1:
                body()
            else:
                with tc.For_i(0, reps, 1) as _i:
                    body(_i)
    nc.compile()
    return nc


def _get(name, builder, reps=1):
    key = (name, reps)
    if key not in _CACHE:
        _CACHE[key] = builder(reps)
    return _CACHE[key]


def run_phase1(x_img, Wg, bg, ln_gamma, ln_beta, reps=1):
    nc = _get("p1", _build_phase1, reps)
    wg_np = np.ascontiguousarray(Wg, dtype=np.float32)
    bg_np = np.ascontiguousarray(bg, dtype=np.float32).reshape(1, E)
    g_np = np.ascontiguousarray(ln_gamma, dtype=np.float32).reshape(1, D)
    b_np = np.ascontiguousarray(ln_beta, dtype=np.float32).reshape(1, D)
    in_maps = [{
        "x": np.ascontiguousarray(x_img[i], dtype=np.float32),
        "wg": wg_np, "bg": bg_np, "gamma": g_np, "beta": b_np,
    } for i in range(N_CORES)]
    return run_bass_kernel_spmd(nc, in_maps, core_ids=list(range(N_CORES)))


def run_phase2(in_maps, reps=1):
    nc = _get("p2", _build_phase2, reps)
    return run_bass_kernel_spmd(nc, in_maps, core_ids=list(range(N_CORES)))


def kernel(x_img, ln_gamma, ln_beta, Wg, bg, W1, b1, W2, b2):
    x_img = np.asarray(x_img)
    res1 = run_phase1(x_img, np.asarray(Wg), np.asarray(bg),
                      np.asarray(ln_gamma), np.asarray(ln_beta))
    xnt_all = np.concatenate([res1.results[i]["xnt"] for i in range(N_CORES)],
                             axis=1)                      # [D, T] f32
    combine_all = np.concatenate(
        [res1.results[i]["combine"] for i in range(N_CORES)], axis=0)  # [T, E]
    stats = np.sum([res1.results[i]["stats"] for i in range(N_CORES)], axis=0)

    # host all-to-all dispatch
    xnt_bf = xnt_all.astype(BF16_NP)
    W1 = np.asarray(W1, dtype=np.float32)
    W2 = np.asarray(W2, dtype=np.float32)
    b1 = np.asarray(b1, dtype=np.float32)
    b2 = np.asarray(b2, dtype=np.float32)
    in_maps = []
    idx_parts = []
    for e in range(E):
        idx = np.nonzero(combine_all[:, e] > 0.0)[0]
        assert len(idx) <= 2 * CAP, f"expert {e} overflow: {len(idx)} > {2*CAP}"
        half = (len(idx) + 1) // 2
        w1e = np.ascontiguousarray(W1[e]).astype(BF16_NP)
        w2e = np.ascontiguousarray(W2[e]).astype(BF16_NP)
        b1e = np.ascontiguousarray(b1[e])
        b2e = np.ascontiguousarray(b2[e])
        for part in range(2):
            pidx = idx[:half] if part == 0 else idx[half:]
            idx_parts.append(pidx)
            xt = np.zeros((D, CAP), dtype=BF16_NP)
            xt[:, :len(pidx)] = xnt_bf[:, pidx]
            scv = np.zeros((1, CAP), dtype=np.float32)
            scv[0, :len(pidx)] = combine_all[pidx, e]
            in_maps.append({"xt": xt, "w1": w1e, "b1": b1e,
                            "w2": w2e, "b2": b2e, "sc": scv})

    res2 = run_phase2(in_maps)

    out = np.zeros((T, H), dtype=np.float32)
    for c in range(N_CORES):
        pidx = idx_parts[c]
        if len(pidx):
            out[pidx] += res2.results[c]["ot"][:, :len(pidx)].T

    counts, prb = stats[0], stats[1]
    frac = counts / (T * K)
    mean_p = prb / T
    aux = np.float32(E * np.sum(frac * mean_p))
    return out.reshape(B, N_TOK, H), aux
